# revision 1
# baseline (speedup 1.0000x reference)
"""Trainium2 Bass kernel for DynConv2d (DGCNN-style edge conv).

Reference computation (per batch b of 4):
  feats  = x[b,:,:,0].T                      # [N=8192, C=64]
  nn_idx = top16_j( 2*<f_i,f_j> - |f_i|^2 - |f_j|^2 )    # kNN graph
  edge   = [x_i, x_j - x_i] @ W.T + bias     # 1x1 conv, W [128, 128]
  out    = max over 16 neighbors             # -> [128, N]

Key algebraic reduction used here: with W = [W1 | W2],
  out[n, c] = u_n[c] + max_{j in top16(n)} v_j[c]
  u = (W1 - W2) @ feats.T + bias             # [128, N]
  v = W2 @ feats.T                           # [128, N]
so the per-edge conv disappears; only a per-row top-16 over the key
matrix key[i, j] = <f_i, f_j> - 0.5*|f_j|^2 (same ordering as the
reference's negative squared distance, since the -|f_i|^2 term is
constant per row) and a gather+max over v remain.

Sharding: 8 cores = 4 batches x 2 halves of N. Each core gets the full
feature matrix of its batch (replicated) plus its local half of rows and
produces out[128, 4096]; the host concatenates. No collectives.
"""

import sys

for _p in ("/opt/trn_rl_repo", "/root/.axon_site/_ro/trn_rl_repo"):
    if _p not in sys.path:
        sys.path.insert(0, _p)

import numpy as np

B = 4
CIN = 64
COUT = 128
N = 8192
K = 16
N_CORES = 8

_prog_cache = {}


def build_program(n=N, r=N // 2, num_devices=N_CORES, repeat=1,
                  no_topk=False, no_gather=False, minimal=False,
                  split_keys=False):
    """Build + compile the SPMD bass program (same NEFF on all cores).

    repeat>1 wraps the main loop in a device-side For_i for benchmarking
    (output is rewritten identically each iteration). no_topk/no_gather/
    minimal are benchmarking ablations (wrong results).

    split_keys: compute the key matmul as 3 accumulated bf16 matmuls on
    split operands (xh*yh + xh*yl + xl*yh, with x = xh + xl + O(2^-17 x))
    instead of one 4-pass fp32 matmul. ~2.7x less PE time; key error
    ~2^-17 relative, far below typical top-16 decision gaps."""
    import concourse.bacc as bacc
    import concourse.mybir as mybir
    import concourse.tile as tile

    f32 = mybir.dt.float32
    bf16 = mybir.dt.bfloat16
    i16 = mybir.dt.int16
    u32 = mybir.dt.uint32
    CH = 512
    nch = n // CH
    rt_count = r // 128

    nc = bacc.Bacc("TRN2", target_bir_lowering=False, debug=False,
                   num_devices=num_devices)

    feats_d = nc.dram_tensor("feats", [CIN, n], f32, kind="ExternalInput")
    featsl_d = nc.dram_tensor("featsl", [CIN, r], f32, kind="ExternalInput")
    w2t_d = nc.dram_tensor("w2t", [CIN, COUT], f32, kind="ExternalInput")
    wdt_d = nc.dram_tensor("wdt", [CIN, COUT], f32, kind="ExternalInput")
    bias_d = nc.dram_tensor("bias", [COUT, 1], f32, kind="ExternalInput")
    ident_d = nc.dram_tensor("ident", [128, 128], f32, kind="ExternalInput")
    out_d = nc.dram_tensor("out", [COUT, r], f32, kind="ExternalOutput")

    with tile.TileContext(nc) as tc:
        with tc.tile_pool(name="const", bufs=1) as const, \
             tc.tile_pool(name="keys", bufs=2) as keysp, \
             tc.tile_pool(name="vg", bufs=4) as vgp, \
             tc.tile_pool(name="small", bufs=3) as small, \
             tc.tile_pool(name="psk", bufs=4, space="PSUM") as psk, \
             tc.tile_pool(name="psa", bufs=2, space="PSUM") as psa:

            # ---------------- prologue ----------------
            # fp32 staging lives in borrowed "keys" slots (prologue only);
            # the bf16 split operands (persistent) are in the const pool.
            if split_keys:
                feats_aug = keysp.tile([CIN + 1, n], f32, tag="keys")
                feats_ones = None  # local fp32 slice lives in featsq's slot
            else:
                feats_aug = const.tile([CIN + 1, n], f32)
                feats_ones = const.tile([CIN + 1, r], f32)
                nc.sync.dma_start(feats_ones[0:CIN, :], featsl_d.ap())
                nc.vector.memset(feats_ones[CIN:CIN + 1, :], 1.0)
            nc.sync.dma_start(feats_aug[0:CIN, :], feats_d.ap())

            w2t = const.tile([CIN, COUT], f32)
            nc.sync.dma_start(w2t[:, :], w2t_d.ap())
            wdt = const.tile([CIN, COUT], f32)
            nc.sync.dma_start(wdt[:, :], wdt_d.ap())
            bias = const.tile([COUT, 1], f32)
            nc.sync.dma_start(bias[:, :], bias_d.ap())
            ident = const.tile([128, 128], f32)
            nc.sync.dma_start(ident[:, :], ident_d.ap())
            ones64 = const.tile([CIN, 1], f32)
            nc.vector.memset(ones64[:, :], 1.0)

            vt = const.tile([COUT, n], f32)
            ut = const.tile([COUT, r], f32)

            # |f_j|^2 row: square, then ones-matmul partition sum
            featsq = keysp.tile([CIN + 1, n], f32, tag="keys")
            nc.scalar.activation(featsq[0:CIN, :], feats_aug[0:CIN, :],
                                 mybir.ActivationFunctionType.Square)
            for c in range(nch):
                sl = slice(c * CH, (c + 1) * CH)
                pxx = psa.tile([1, CH], f32, tag="psa")
                nc.tensor.matmul(pxx[:, :], ones64[:, :], featsq[0:CIN, sl],
                                 start=True, stop=True)
                xs = small.tile([1, CH], f32, tag="xs")
                nc.scalar.activation(xs[:, :], pxx[:, :],
                                     mybir.ActivationFunctionType.Copy, scale=-0.5)
                # DMA shifts partition base: row 64 of feats_aug = -0.5*xx
                nc.sync.dma_start(feats_aug[CIN:CIN + 1, sl], xs[:, :])

            # v = W2 @ feats.T   -> [128, n]
            for c in range(nch):
                sl = slice(c * CH, (c + 1) * CH)
                pv = psa.tile([COUT, CH], f32, tag="psa")
                nc.tensor.matmul(pv[:, :], w2t[:, :], feats_aug[0:CIN, sl],
                                 start=True, stop=True)
                nc.scalar.copy(vt[:, sl], pv[:, :])

            # u = (W1-W2) @ featsl.T + bias  -> [128, r]
            if split_keys:
                # featsq's slot doubles as the fp32 local-rows staging
                featsl_f32 = featsq
                nc.sync.dma_start(featsl_f32[0:CIN, 0:r], featsl_d.ap())
            else:
                featsl_f32 = feats_ones
            for c in range(r // CH):
                sl = slice(c * CH, (c + 1) * CH)
                pu = psa.tile([COUT, CH], f32, tag="psa")
                nc.tensor.matmul(pu[:, :], wdt[:, :], featsl_f32[0:CIN, sl],
                                 start=True, stop=True)
                nc.vector.tensor_scalar_add(ut[:, sl], pu[:, :], bias[:, :])

            if split_keys:
                # bf16 split operands: x = H + L + O(2^-17 x)
                augH = const.tile([CIN + 1, n], bf16)
                augL = const.tile([CIN + 1, n], bf16)
                oneH = const.tile([CIN + 1, r], bf16)
                oneL = const.tile([CIN + 1, r], bf16)
                nc.vector.tensor_copy(augH[:, :], feats_aug[:, :])
                nc.vector.tensor_sub(augL[:, :], feats_aug[:, :], augH[:, :])
                nc.scalar.copy(oneH[0:CIN, :], featsl_f32[0:CIN, 0:r])
                nc.vector.memset(oneH[CIN:CIN + 1, :], 1.0)
                nc.vector.memset(oneL[CIN:CIN + 1, :], 0.0)
                nc.vector.tensor_sub(oneL[0:CIN, :], featsl_f32[0:CIN, 0:r],
                                     oneH[0:CIN, :])

            # ---------------- main loop over row tiles ----------------
            def main_body():
                for rt in range(rt_count):
                    tile_body(rt)

            def tile_body(rt):
                rsl = slice(rt * 128, (rt + 1) * 128)
                keys = keysp.tile([128, n], f32, tag="keys")
                for c in range(nch):
                    sl = slice(c * CH, (c + 1) * CH)
                    pk = psk.tile([128, CH], f32, tag="psk")
                    if split_keys:
                        nc.tensor.matmul(pk[:, :], oneH[:, rsl], augH[:, sl],
                                         start=True, stop=False)
                        nc.tensor.matmul(pk[:, :], oneH[:, rsl], augL[:, sl],
                                         start=False, stop=False)
                        nc.tensor.matmul(pk[:, :], oneL[:, rsl], augH[:, sl],
                                         start=False, stop=True)
                    else:
                        nc.tensor.matmul(pk[:, :], feats_ones[:, rsl],
                                         feats_aug[:, sl], start=True, stop=True)
                    nc.scalar.copy(keys[:, sl], pk[:, :])

                if minimal:
                    ot0 = small.tile([128, 128], f32, tag="ot")
                    nc.vector.tensor_add(ot0[:, :], keys[:, 0:128], ut[:, rsl])
                    nc.sync.dma_start(out_d.ap()[:, rsl], ot0[:, :])
                    return

                jf = small.tile([128, 16], f32, tag="jf")
                if no_topk:
                    nc.vector.memset(jf[:, :], 5.0)
                else:
                    # top-16 (values + global column indices) per row
                    r1 = small.tile([128, 8], f32, tag="r8")
                    nc.vector.max(r1[:, :], keys[:, :])
                    i1 = small.tile([128, 8], u32, tag="i8")
                    nc.vector.max_index(i1[:, :], r1[:, :], keys[:, :])
                    nc.vector.match_replace(keys[:, :], r1[:, :], keys[:, :],
                                            -3.0e38)
                    r2 = small.tile([128, 8], f32, tag="r8")
                    nc.vector.max(r2[:, :], keys[:, :])
                    i2 = small.tile([128, 8], u32, tag="i8")
                    nc.vector.max_index(i2[:, :], r2[:, :], keys[:, :])

                    nc.scalar.copy(jf[:, 0:8], i1[:, :])
                    nc.scalar.copy(jf[:, 8:16], i2[:, :])

                # wrapped int16 index layout for ap_gather:
                # widx[16g + q, m] = j[m, q] for all 8 groups g
                tp = psa.tile([16, 128], f32, tag="tp")
                nc.tensor.transpose(tp[:, :], jf[:, :], ident[:, :])
                tpi = small.tile([16, 128], i16, tag="tpi")
                nc.scalar.copy(tpi[:, :], tp[:, :])
                widx = small.tile([128, 128], i16, tag="widx")
                for g in range(8):
                    nc.sync.dma_start(widx[16 * g:16 * (g + 1), :], tpi[:, :])

                mx = small.tile([128, 128], f32, tag="mx")
                if no_gather:
                    nc.vector.tensor_copy(mx[:, :], ut[:, rsl])
                else:
                    # gather v columns of the 2048 neighbors, grouped max
                    vg = vgp.tile([128, 2048], f32, tag="vg")
                    nc.gpsimd.ap_gather(vg[:, :], vt[:, :], widx[:, :],
                                        channels=128, num_elems=n, d=1,
                                        num_idxs=2048)
                    nc.vector.reduce_max(mx[:, :],
                                         vg[:, :].rearrange("p (g k) -> p g k",
                                                            k=K),
                                         axis=mybir.AxisListType.X)
                ot = small.tile([128, 128], f32, tag="ot")
                nc.vector.tensor_add(ot[:, :], mx[:, :], ut[:, rsl])
                nc.sync.dma_start(out_d.ap()[:, rsl], ot[:, :])

            if repeat > 1:
                with tc.For_i(0, repeat, 1):
                    main_body()
            else:
                main_body()

    nc.compile()
    return nc


def _get_program(n, r, num_devices):
    key = (n, r, num_devices)
    if key not in _prog_cache:
        _prog_cache[key] = build_program(n, r, num_devices)
    return _prog_cache[key]


def run_cores(feats_by_core, featsl_by_core, W, b, n, r, trace=False):
    """Run the SPMD program. feats_by_core[i]: [64, n]; featsl_by_core[i]: [64, r]."""
    from concourse.bass_utils import run_bass_kernel_spmd

    num = len(feats_by_core)
    W1 = W[:, :CIN]
    W2 = W[:, CIN:]
    w2t = np.ascontiguousarray(W2.T).astype(np.float32)
    wdt = np.ascontiguousarray((W1 - W2).T).astype(np.float32)
    bias = b.reshape(COUT, 1).astype(np.float32)
    ident = np.eye(128, dtype=np.float32)
    in_maps = []
    for i in range(num):
        in_maps.append({
            "feats": np.ascontiguousarray(feats_by_core[i], dtype=np.float32),
            "featsl": np.ascontiguousarray(featsl_by_core[i], dtype=np.float32),
            "w2t": w2t, "wdt": wdt, "bias": bias, "ident": ident,
        })
    nc = _get_program(n, r, num)
    res = run_bass_kernel_spmd(nc, in_maps, core_ids=list(range(num)), trace=trace)
    return [res.results[i]["out"] for i in range(num)], res


def kernel(x, W, b):
    """Full-input entry point: x [4, 64, 8192, 1] f32 -> [4, 128, 8192, 1] f32."""
    x = np.asarray(x, dtype=np.float32)
    W = np.asarray(W, dtype=np.float32)
    b = np.asarray(b, dtype=np.float32)
    xb = np.ascontiguousarray(x[:, :, :, 0])            # [4, 64, 8192]
    r = N // 2
    feats_by_core = []
    featsl_by_core = []
    for core in range(N_CORES):
        bi, half = core // 2, core % 2
        feats_by_core.append(xb[bi])
        featsl_by_core.append(xb[bi][:, half * r:(half + 1) * r])
    outs, _ = run_cores(feats_by_core, featsl_by_core, W, b, N, r)
    out = np.empty((B, COUT, N, 1), np.float32)
    for core in range(N_CORES):
        bi, half = core // 2, core % 2
        out[bi, :, half * r:(half + 1) * r, 0] = outs[core]
    return out

